# revision 6
# baseline (speedup 1.0000x reference)
"""DSFusion kernel for 8x TRN2 NeuronCores.

Computation (per reference):
    out_x = x @ Wx.T + bx ; out_y = y @ Wy.T + by
    sp1 = softplus(out_x) ; sp2 = softplus(out_y)
    alpha_x = sp1 + 1 ; alpha_y = sp2 + 1
    alpha_a = sp1*sp2/C + sp1 + sp2 + 1        (algebraic collapse of the
                                                Dempster-Shafer combination --
                                                all S/b/u/conflict terms cancel)
            = sp1*(sp2/C + 1) + alpha_y

Sharding: data-parallel over the batch dim, 1024 rows per core; weights and
biases replicated. Host pre-transposes x/y/W so the contraction dim sits on
SBUF partitions and pre-casts matmul operands to bf16 (fp32 PSUM accumulate).

Schedule: every matmul phase is split by class half (500 cols = one PSUM
bank) and runs k-outer over a group of row tiles, so a phase of G tiles
occupies G banks and each bank gets a full phase (>=3.3us) to drain before
reuse -- no PSUM stalls at phase boundaries. Units of [4,2,1,1] row tiles:
the 4-tile first unit stretches the first X phase to ~27us so the wx bulk
(4.1MB) amortizes under the 358GB/s DMA roofline while wy/y prefetch in the
slack. Input DMA triggers cost ~600ns of sequencer time each and are
in-order per engine, so they live only on sequencers with no mid-run
compute: SP takes the phase-0/1 ramp (wx, x0, biases) plus all output
DMAs, Pool takes the later bulk (wy, y, x1). Gating (add_dep_helper) on PE
k-progress keeps bulk transfers from starving the just-in-time ramp
chunks. The last unit's Y phase is quarter-split with per-quarter
epilogues so only a ~250-wide chain is exposed after the final matmul.
"""

import numpy as np
import ml_dtypes

BATCH = 8192
DIM = 2048
CLASSES = 1000
NCORES = 8
R = BATCH // NCORES          # rows per core (1024)
P = 128
KCH = DIM // P               # contraction chunks (16)
NT = R // P                  # row tiles per core (8)
NH = CLASSES // 2            # class half (500, fits one 2KB psum bank)
QN = CLASSES // 4            # class quarter (250)

_CACHE = {}

# Results of the last device run (for the test harness to inspect timing).
LAST_RESULTS = None


def _split_waits(nc, limit=1):
    """The installed walrus can't lower an instruction carrying more than one
    sync wait. Hoist extra waits onto single-wait NOPs inserted immediately
    before the instruction on the same engine (program order preserves the
    wait-all semantics)."""
    import concourse.mybir as mybir

    for f in nc.m.functions:
        for bb in f.blocks:
            out = []
            changed = False
            for ins in list(bb.instructions):
                si = ins.sync_info
                if si is not None and len(si.on_wait) > limit:
                    waits = list(si.on_wait)
                    extra, keep = waits[:-limit], waits[-limit:]
                    for i, w in enumerate(extra):
                        nop = mybir.InstNoOp(name=f"{ins.name}-ws{i}", ins=[], outs=[])
                        nop.engine = ins.engine
                        nop.sync_info = mybir.SyncInfo(on_wait=[w], on_update=[])
                        out.append(nop)
                    ins.sync_info = mybir.SyncInfo(
                        on_wait=keep, on_update=list(si.on_update)
                    )
                    changed = True
                out.append(ins)
            if changed:
                bb.instructions = out


def _build_nc():
    import concourse.bass as bass
    import concourse.mybir as mybir
    import concourse.tile as tile
    from concourse.vector_clock import ScopedClock, VectorClock

    class LeanTailTileContext(tile.TileContext):
        """Tile's stock tail is drain + two all-engine barriers + sem clears;
        with the single-wait-per-instruction legalization the barrier waits
        explode into a serial EVSEM parade. Replace with: per-proc drain
        waits spread round-robin over five engines (run in parallel), a
        two-semaphore handshake barrier (one wait per engine), then gpsimd
        range-clears everything last."""

        def _drain_and_barrier(self, tick_clock, wait_clock):
            nc = self.nc
            vc = tick_clock.global_clock
            n = len(vc)
            wait_engines = [nc.sync, nc.scalar, nc.vector, nc.gpsimd, nc.tensor]
            wi = 0
            for proc in range(n):
                t = vc[proc]
                if t > 0:
                    eng = wait_engines[wi % len(wait_engines)]
                    wi += 1
                    nop = eng.nop(nofuse=True, hint=f"tail_wait_{proc}")
                    req = ScopedClock(
                        {None: VectorClock([t if i == proc else 0 for i in range(n)])}
                    )
                    wait_clock.add_sem_waits(nop.ins, req)
            nc.sync.drain()

            semB = nc.alloc_semaphore("tail_barrier_b")
            semC = nc.alloc_semaphore("tail_barrier_c")
            engines = list(nc.engines.values())
            pool_eng = nc.gpsimd
            n_eng = len(engines)
            for e in engines:
                e.nop(nofuse=True, hint="tailb_inc").then_inc(semB, 1)
            for e in engines:
                e.wait_ge(semB, n_eng)
            for e in engines:
                if e is not pool_eng:
                    e.nop(nofuse=True, hint="tailc_inc").then_inc(semC, 1)
            pool_eng.wait_ge(semC, n_eng - 1)

            assert self.sems is not None
            popped = self.nc._tile_sem_poison_stack.pop()
            assert popped is self._sem_poison
            nc.clear_and_free_semaphores(
                list(self.sems.allocated().values()) + [semB, semC]
            )

    dt = mybir.dt

    nc = bass.Bass()

    xT = nc.dram_tensor("xT", [DIM, R], dt.bfloat16, kind="ExternalInput")
    yT = nc.dram_tensor("yT", [DIM, R], dt.bfloat16, kind="ExternalInput")
    wxT = nc.dram_tensor("wxT", [DIM, CLASSES], dt.bfloat16, kind="ExternalInput")
    wyT = nc.dram_tensor("wyT", [DIM, CLASSES], dt.bfloat16, kind="ExternalInput")
    bxb = nc.dram_tensor("bxb", [P, CLASSES], dt.bfloat16, kind="ExternalInput")
    byb = nc.dram_tensor("byb", [P, CLASSES], dt.bfloat16, kind="ExternalInput")

    aa_d = nc.dram_tensor("alpha_a", [R, CLASSES], dt.float32, kind="ExternalOutput")
    ax_d = nc.dram_tensor("alpha_x", [R, CLASSES], dt.float32, kind="ExternalOutput")
    ay_d = nc.dram_tensor("alpha_y", [R, CLASSES], dt.float32, kind="ExternalOutput")

    xT3 = xT.rearrange("(ko p) r -> p ko r", p=P)
    yT3 = yT.rearrange("(ko p) r -> p ko r", p=P)
    wxT3 = wxT.rearrange("(ko p) c -> p ko c", p=P)
    wyT3 = wyT.rearrange("(ko p) c -> p ko c", p=P)
    aa3 = aa_d.rearrange("(t p) c -> t p c", p=P)
    ax3 = ax_d.rearrange("(t p) c -> t p c", p=P)
    ay3 = ay_d.rearrange("(t p) c -> t p c", p=P)

    # softplus(x) = ln(exp(x) + 1); the installed ACT tables have no direct
    # softplus, but exp and ln share one table set. Pre-activation values are
    # within +-4 so exp cannot overflow.
    EXP = mybir.ActivationFunctionType.Exp
    LN = mybir.ActivationFunctionType.Ln
    ADD = mybir.AluOpType.add
    MULT = mybir.AluOpType.mult

    HS = [slice(0, NH), slice(NH, CLASSES)]

    with LeanTailTileContext(nc) as tc:
        with (
            tc.tile_pool(name="wpool", bufs=1) as wpool,
            tc.tile_pool(name="xpool", bufs=1) as xpool,
            tc.tile_pool(name="spool", bufs=1) as spool,
            tc.tile_pool(name="opool", bufs=1) as opool,
            tc.tile_pool(name="psum", bufs=1, space="PSUM") as ppool,
        ):
            # -- input DMAs ------------------------------------------------
            # One kk-chunk (2 K slices) per DMA so the PE's wait granularity
            # matches arrival. SP programs the phase-0 ramp in consumption
            # order; Pool programs everything that fires from phase 1 on.
            wx_sb = [[None] * (KCH // 2) for _ in range(2)]   # [h][kk]
            wy_sb = [[None] * (KCH // 2) for _ in range(2)]
            wx_dma = [[None] * (KCH // 2) for _ in range(2)]
            wy_dma = [[None] * (KCH // 2) for _ in range(2)]
            x0_sb, x1_sb, y0_sb, y1_sb = [], [], [], []
            x0_dma, x1_dma, y0_dma, y1_dma = [], [], [], []

            # The first 4 K slices ride single-k DMAs so the very first
            # matmul's data is ~128KB away, not ~512KB; wx ramp on SP and
            # x ramp on Pool program in parallel.
            wx0s_sb, wx0s_dma, x0s_sb, x0s_dma = [], [], [], []
            for k in range(4):
                t_ = wpool.tile([P, 1, NH], dt.bfloat16, tag=f"wx0s_{k}")
                wx0s_dma.append(nc.sync.dma_start(t_[:], wxT3[:, k:k + 1, HS[0]]))
                wx0s_sb.append(t_)
            for kk in range(2, KCH // 2):
                t_ = wpool.tile([P, 2, NH], dt.bfloat16, tag=f"wx0_{kk}")
                wx_dma[0][kk] = nc.sync.dma_start(
                    t_[:], wxT3[:, 2 * kk:2 * kk + 2, HS[0]]
                )
                wx_sb[0][kk] = t_
            for kk in range(KCH // 2):
                t_ = wpool.tile([P, 2, NH], dt.bfloat16, tag=f"wx1_{kk}")
                wx_dma[1][kk] = nc.sync.dma_start(
                    t_[:], wxT3[:, 2 * kk:2 * kk + 2, HS[1]]
                )
                wx_sb[1][kk] = t_
            bx_sb = wpool.tile([P, CLASSES], dt.bfloat16, tag="bx")
            bx_dma = nc.sync.dma_start(bx_sb[:], bxb[:])

            # Pool, in firing order: x0 ramp (phase 0), wy-h0 + y0
            # (phase 1), wy-h1 + x1 (phase 2), y1 (phase 3).
            for k in range(4):
                t_ = xpool.tile([P, 1, 4 * P], dt.bfloat16, tag=f"x0s_{k}")
                x0s_dma.append(nc.gpsimd.dma_start(t_[:], xT3[:, k:k + 1, 0:4 * P]))
                x0s_sb.append(t_)
            for kk in range(2, KCH // 2):
                t_ = xpool.tile([P, 2, 4 * P], dt.bfloat16, tag=f"x0_{kk}")
                x0_dma.append(nc.gpsimd.dma_start(t_[:], xT3[:, 2 * kk:2 * kk + 2, 0:4 * P]))
                x0_sb.append(t_)
            for kk in range(KCH // 2):
                t_ = wpool.tile([P, 2, NH], dt.bfloat16, tag=f"wy0_{kk}")
                wy_dma[0][kk] = nc.gpsimd.dma_start(
                    t_[:], wyT3[:, 2 * kk:2 * kk + 2, HS[0]]
                )
                wy_sb[0][kk] = t_
                t_ = xpool.tile([P, 2, 4 * P], dt.bfloat16, tag=f"y0_{kk}")
                y0_dma.append(nc.gpsimd.dma_start(t_[:], yT3[:, 2 * kk:2 * kk + 2, 0:4 * P]))
                y0_sb.append(t_)
            by_sb = wpool.tile([P, CLASSES], dt.bfloat16, tag="by")
            by_dma = nc.gpsimd.dma_start(by_sb[:], byb[:])
            for kk in range(KCH // 2):
                t_ = wpool.tile([P, 2, NH], dt.bfloat16, tag=f"wy1_{kk}")
                wy_dma[1][kk] = nc.gpsimd.dma_start(
                    t_[:], wyT3[:, 2 * kk:2 * kk + 2, HS[1]]
                )
                wy_sb[1][kk] = t_
                t_ = xpool.tile([P, 2, 4 * P], dt.bfloat16, tag=f"x1_{kk}")
                x1_dma.append(nc.gpsimd.dma_start(t_[:], xT3[:, 2 * kk:2 * kk + 2, 4 * P:R]))
                x1_sb.append(t_)
            for kk in range(KCH // 2):
                t_ = xpool.tile([P, 2, 4 * P], dt.bfloat16, tag=f"y1_{kk}")
                y1_dma.append(nc.gpsimd.dma_start(t_[:], yT3[:, 2 * kk:2 * kk + 2, 4 * P:R]))
                y1_sb.append(t_)

            def x_slice(k, r):  # lhsT for global row tile r, K-chunk k
                if r < 4:
                    if k < 4:
                        return x0s_sb[k][:, 0, r * P:(r + 1) * P]
                    return x0_sb[k // 2 - 2][:, k % 2, r * P:(r + 1) * P]
                return x1_sb[k // 2][:, k % 2, (r - 4) * P:(r - 3) * P]

            def y_slice(k, r):
                if r < 4:
                    return y0_sb[k // 2][:, k % 2, r * P:(r + 1) * P]
                return y1_sb[k // 2][:, k % 2, (r - 4) * P:(r - 3) * P]

            def wx_slice(k, h, cs=slice(0, NH)):
                if h == 0 and k < 4:
                    return wx0s_sb[k][:, 0, cs]
                return wx_sb[h][k // 2][:, k % 2, cs]

            def wy_slice(k, h, cs=slice(0, NH)):
                return wy_sb[h][k // 2][:, k % 2, cs]

            # PE warm-up: the PE idles at a low p-state and ramps with
            # activity; dummy matmuls during the DMA ramp bring it to full
            # clock before the first real matmul.
            wl = spool.tile([P, P], dt.bfloat16, tag="warm_l")
            nc.vector.memset(wl[:], 0)
            wr = spool.tile([P, NH], dt.bfloat16, tag="warm_r")
            nc.vector.memset(wr[:], 0)
            wp = ppool.tile([P, NH], dt.float32, tag="bk7", name="warmp")
            for _ in range(9):
                nc.tensor.matmul(wp[:], wl[:], wr[:], start=True, stop=True)

            # -- compute ---------------------------------------------------
            # Units of [4,2,1,1] row tiles; per unit the phases are
            # X-h0, X-h1, Y-h0, Y-h1, each k-outer over the unit's tiles.
            # Each phase's epilogue overlaps the next phase's matmuls.
            from concourse.tile_rust import add_dep_helper

            UNITS = [(0, 4), (4, 2), (6, 1), (7, 1)]
            mm_anchor = {}  # (phase_idx, k) -> last MM instruction
            bank_ptr = 0    # rotating PSUM bank pointer
            phase_idx = 0

            # t1 holds sp1 = softplus(out_x) per row tile from X epilogue to
            # Y epilogue. Tags: per-jj for U0 (reused by U1), own for U2/U3.
            t1_t = [None] * NT

            def t1_tag(u, r):
                return f"t1_{r % 4}" if u < 2 else f"t1_u{u}"

            for u, (r0, nrt) in enumerate(UNITS):
                last_unit = u == len(UNITS) - 1

                # ---- X phases (h = 0, 1) ----
                psx = [[None] * nrt for _ in range(2)]
                for h in range(2):
                    for jj in range(nrt):
                        b = (bank_ptr + jj) % 8
                        psx[h][jj] = ppool.tile(
                            [P, NH], dt.float32, tag=f"bk{b}", name=f"xu{u}h{h}j{jj}"
                        )
                    bank_ptr += nrt
                    for k in range(KCH):
                        st, sp = k == 0, k == KCH - 1
                        for jj in range(nrt):
                            mm = nc.tensor.matmul(
                                psx[h][jj][:], x_slice(k, r0 + jj), wx_slice(k, h),
                                start=st, stop=sp,
                            )
                        mm_anchor[(phase_idx, k)] = mm.ins
                    phase_idx += 1

                    # h epilogue (overlaps the next phase's matmuls):
                    # sp1 = softplus(psum + bx) -> t1; ax = sp1+1 -> DMA.
                    for jj in range(nrt):
                        r = r0 + jj
                        if h == 0:
                            t1_t[r] = spool.tile(
                                [P, CLASSES], dt.float32, tag=t1_tag(u, r),
                                name=f"t1_{r}",
                            )
                        hs = HS[h]
                        sp1 = t1_t[r][:, hs]
                        nc.vector.tensor_tensor(sp1, psx[h][jj][:], bx_sb[:, hs], ADD)
                        nc.scalar.activation(sp1, sp1, EXP)
                        nc.scalar.activation(sp1, sp1, LN, bias=1.0)
                        axt = opool.tile([P, CLASSES], dt.float32, tag=f"ax_{jj % 2}")
                        nc.vector.tensor_scalar_add(axt[:, hs], sp1, 1.0)
                        nc.sync.dma_start(ax3[r][:, hs], axt[:, hs])

                # ---- Y phases ----
                def y_epilogue(r, psum, cs, t2, ay, aa, wslice_unused=None):
                    # sp2 = softplus(psum + by); ay = sp2+1 (ACT) in
                    # parallel with aa = sp1*(sp2/C + 1) + ay (DVE).
                    sp2 = t2[:, cs]
                    nc.vector.tensor_tensor(sp2, psum, by_sb[:, cs], ADD)
                    nc.scalar.activation(sp2, sp2, EXP)
                    nc.scalar.activation(sp2, sp2, LN, bias=1.0)
                    nc.scalar.add(ay[:, cs], sp2, 1.0)
                    nc.sync.dma_start(ay3[r][:, cs], ay[:, cs])
                    w2 = aa[:, cs]
                    nc.vector.tensor_scalar(w2, sp2, 1.0 / CLASSES, 1.0, MULT, ADD)
                    nc.vector.tensor_tensor(w2, w2, t1_t[r][:, cs], MULT)
                    nc.vector.tensor_tensor(w2, w2, ay[:, cs], ADD)
                    nc.sync.dma_start(aa3[r][:, cs], aa[:, cs])

                if last_unit:
                    # Quarter-split with per-quarter epilogue so only a
                    # 250-wide chain is exposed after the last matmul.
                    r = r0
                    t2 = opool.tile([P, CLASSES], dt.float32, tag="t2_0")
                    ay = opool.tile([P, CLASSES], dt.float32, tag="ay_0")
                    aa = opool.tile([P, CLASSES], dt.float32, tag="aa_0")
                    SEGS = [(0, 250), (250, 250), (500, 250), (750, 125), (875, 125)]
                    for q, (s0, w) in enumerate(SEGS):
                        qs = slice(s0, s0 + w)
                        h, off = s0 // NH, s0 % NH
                        b = bank_ptr % 8
                        bank_ptr += 1
                        psq = ppool.tile([P, w], dt.float32, tag=f"bk{b}", name=f"yq{q}")
                        wqs = slice(off, off + w)
                        for k in range(KCH):
                            st, sp = k == 0, k == KCH - 1
                            mm = nc.tensor.matmul(
                                psq[:], y_slice(k, r), wy_slice(k, h, wqs),
                                start=st, stop=sp,
                            )
                        mm_anchor[(phase_idx, k)] = mm.ins
                        phase_idx += 1
                        y_epilogue(r, psq[:], qs, t2, ay, aa)
                    continue

                psy = [[None] * nrt for _ in range(2)]
                for h in range(2):
                    for jj in range(nrt):
                        b = (bank_ptr + jj) % 8
                        psy[h][jj] = ppool.tile(
                            [P, NH], dt.float32, tag=f"bk{b}", name=f"yu{u}h{h}j{jj}"
                        )
                    bank_ptr += nrt
                    for k in range(KCH):
                        st, sp = k == 0, k == KCH - 1
                        for jj in range(nrt):
                            mm = nc.tensor.matmul(
                                psy[h][jj][:], y_slice(k, r0 + jj), wy_slice(k, h),
                                start=st, stop=sp,
                            )
                        mm_anchor[(phase_idx, k)] = mm.ins
                    phase_idx += 1

                    for jj in range(nrt):
                        r = r0 + jj
                        t2 = opool.tile([P, CLASSES], dt.float32, tag=f"t2_{jj % 2}")
                        ay = opool.tile([P, CLASSES], dt.float32, tag=f"ay_{jj % 2}")
                        aa = opool.tile([P, CLASSES], dt.float32, tag=f"aa_{jj % 2}")
                        y_epilogue(r, psy[h][jj][:], HS[h], t2, ay, aa)

            # -- DMA backpressure: gate transfers on PE progress -----------
            # Phase ids: U0: 0=X-h0 1=X-h1 2=Y-h0 3=Y-h1; U1: 4..7;
            # U2: 8..11; U3: 12,13 then quarters 14..17.
            def _gate(dma, phase, k, why):
                add_dep_helper(
                    dma.ins, mm_anchor[(phase, min(max(k, 0), KCH - 1))], reason=why
                )

            for kk in range(3, KCH // 2):
                # pairs cover k=2kk..2kk+1; release ~7 k-slots ahead
                _gate(wx_dma[0][kk], 0, 2 * kk - 7, "wx h0 ramp")
                _gate(x0_dma[kk - 2], 0, 2 * kk - 7, "x u0 ramp")
            for kk in range(KCH // 2):
                _gate(wx_dma[1][kk], 0, min(2 * kk + 4, KCH - 1), "wx h1 stage")
                _gate(wy_dma[0][kk], 1, min(2 * kk + 2, KCH - 1), "wy h0 stage")
                _gate(y0_dma[kk], 1, min(2 * kk + 2, KCH - 1), "y u0 stage")
                _gate(wy_dma[1][kk], 2, min(2 * kk + 2, KCH - 1), "wy h1 stage")
                _gate(x1_dma[kk], 2, 2 * kk, "x rest stage")
                _gate(y1_dma[kk], 3, 2 * kk, "y rest stage")
            _gate(bx_dma, 0, 10, "bias x stage")
            _gate(by_dma, 1, 10, "bias y stage")

    _split_waits(nc)
    return nc


def kernel(x, y, Wx, bx, Wy, by):
    global LAST_RESULTS
    from concourse.bass_utils import run_bass_kernel_spmd

    if "nc" not in _CACHE:
        _CACHE["nc"] = _build_nc()
    nc = _CACHE["nc"]

    bf16 = ml_dtypes.bfloat16
    x = np.asarray(x, dtype=np.float32)
    y = np.asarray(y, dtype=np.float32)
    xb = x.astype(bf16)                       # [BATCH, DIM]
    yb = y.astype(bf16)
    wxT = np.ascontiguousarray(np.asarray(Wx, dtype=np.float32).astype(bf16).T)  # [DIM, CLASSES]
    wyT = np.ascontiguousarray(np.asarray(Wy, dtype=np.float32).astype(bf16).T)
    bxb = np.ascontiguousarray(
        np.broadcast_to(np.asarray(bx, dtype=np.float32).astype(bf16), (P, CLASSES))
    )
    byb = np.ascontiguousarray(
        np.broadcast_to(np.asarray(by, dtype=np.float32).astype(bf16), (P, CLASSES))
    )

    xTb = np.ascontiguousarray(xb.T)          # [DIM, BATCH]
    yTb = np.ascontiguousarray(yb.T)

    in_maps = []
    for c in range(NCORES):
        rs = slice(c * R, (c + 1) * R)
        in_maps.append(
            {
                "xT": np.ascontiguousarray(xTb[:, rs]),
                "yT": np.ascontiguousarray(yTb[:, rs]),
                "wxT": wxT,
                "wyT": wyT,
                "bxb": bxb,
                "byb": byb,
            }
        )

    res = run_bass_kernel_spmd(nc, in_maps, core_ids=list(range(NCORES)))
    LAST_RESULTS = res

    aa = np.concatenate([res.results[c]["alpha_a"] for c in range(NCORES)], axis=0)
    ax = np.concatenate([res.results[c]["alpha_x"] for c in range(NCORES)], axis=0)
    ay = np.concatenate([res.results[c]["alpha_y"] for c in range(NCORES)], axis=0)
    return (aa, ax, ay)


# revision 9
# speedup vs baseline: 1.0198x; 1.0198x over previous
"""DSFusion kernel for 8x TRN2 NeuronCores.

Computation (per reference):
    out_x = x @ Wx.T + bx ; out_y = y @ Wy.T + by
    sp1 = softplus(out_x) ; sp2 = softplus(out_y)
    alpha_x = sp1 + 1 ; alpha_y = sp2 + 1
    alpha_a = sp1*sp2/C + sp1 + sp2 + 1        (algebraic collapse of the
                                                Dempster-Shafer combination --
                                                all S/b/u/conflict terms cancel)
            = sp1*(sp2/C + 1) + alpha_y

Sharding: data-parallel over the batch dim, 1024 rows per core; weights and
biases replicated. Host pre-transposes x/y/W so the contraction dim sits on
SBUF partitions and pre-casts matmul operands to bf16 (fp32 PSUM accumulate).

Schedule: every matmul phase is split by class half (500 cols = one PSUM
bank) and runs k-outer over a group of row tiles, so a phase of G tiles
occupies G banks and each bank gets a full phase (>=3.3us) to drain before
reuse -- no PSUM stalls at phase boundaries. Units of [4,2,1,1] row tiles:
the 4-tile first unit stretches the first X phase to ~27us so the wx bulk
(4.1MB) amortizes under the 358GB/s DMA roofline while wy/y prefetch in the
slack. Input DMA triggers cost ~600ns of sequencer time each and are
in-order per engine, so they live only on sequencers with no mid-run
compute: SP takes the phase-0/1 ramp (wx, x0, biases) plus all output
DMAs, Pool takes the later bulk (wy, y, x1). Gating (add_dep_helper) on PE
k-progress keeps bulk transfers from starving the just-in-time ramp
chunks. The last unit's Y phase is quarter-split with per-quarter
epilogues so only a ~250-wide chain is exposed after the final matmul.
"""

import numpy as np
import ml_dtypes

BATCH = 8192
DIM = 2048
CLASSES = 1000
NCORES = 8
R = BATCH // NCORES          # rows per core (1024)
P = 128
KCH = DIM // P               # contraction chunks (16)
NT = R // P                  # row tiles per core (8)
NH = CLASSES // 2            # class half (500, fits one 2KB psum bank)
QN = CLASSES // 4            # class quarter (250)

_CACHE = {}

# Results of the last device run (for the test harness to inspect timing).
LAST_RESULTS = None


def _split_waits(nc, limit=1):
    """The installed walrus can't lower an instruction carrying more than one
    sync wait. Hoist extra waits onto single-wait NOPs inserted immediately
    before the instruction on the same engine (program order preserves the
    wait-all semantics)."""
    import concourse.mybir as mybir

    for f in nc.m.functions:
        for bb in f.blocks:
            out = []
            changed = False
            for ins in list(bb.instructions):
                si = ins.sync_info
                if si is not None and len(si.on_wait) > limit:
                    waits = list(si.on_wait)
                    extra, keep = waits[:-limit], waits[-limit:]
                    for i, w in enumerate(extra):
                        nop = mybir.InstNoOp(name=f"{ins.name}-ws{i}", ins=[], outs=[])
                        nop.engine = ins.engine
                        nop.sync_info = mybir.SyncInfo(on_wait=[w], on_update=[])
                        out.append(nop)
                    ins.sync_info = mybir.SyncInfo(
                        on_wait=keep, on_update=list(si.on_update)
                    )
                    changed = True
                out.append(ins)
            if changed:
                bb.instructions = out


def _build_nc():
    import concourse.bass as bass
    import concourse.mybir as mybir
    import concourse.tile as tile
    from concourse.vector_clock import ScopedClock, VectorClock

    class LeanTailTileContext(tile.TileContext):
        """Tile's stock tail is drain + two all-engine barriers + sem clears;
        with the single-wait-per-instruction legalization the barrier waits
        explode into a serial EVSEM parade. Replace with: per-proc drain
        waits spread round-robin over five engines (run in parallel), a
        two-semaphore handshake barrier (one wait per engine), then gpsimd
        range-clears everything last."""

        def _drain_and_barrier(self, tick_clock, wait_clock):
            nc = self.nc
            vc = tick_clock.global_clock
            n = len(vc)
            wait_engines = [nc.sync, nc.scalar, nc.vector, nc.gpsimd, nc.tensor]
            wi = 0
            for proc in range(n):
                t = vc[proc]
                if t > 0:
                    eng = wait_engines[wi % len(wait_engines)]
                    wi += 1
                    nop = eng.nop(nofuse=True, hint=f"tail_wait_{proc}")
                    req = ScopedClock(
                        {None: VectorClock([t if i == proc else 0 for i in range(n)])}
                    )
                    wait_clock.add_sem_waits(nop.ins, req)
            nc.sync.drain()

            semB = nc.alloc_semaphore("tail_barrier_b")
            semC = nc.alloc_semaphore("tail_barrier_c")
            engines = list(nc.engines.values())
            pool_eng = nc.gpsimd
            n_eng = len(engines)
            for e in engines:
                e.nop(nofuse=True, hint="tailb_inc").then_inc(semB, 1)
            for e in engines:
                e.wait_ge(semB, n_eng)
            for e in engines:
                if e is not pool_eng:
                    e.nop(nofuse=True, hint="tailc_inc").then_inc(semC, 1)
            pool_eng.wait_ge(semC, n_eng - 1)

            assert self.sems is not None
            popped = self.nc._tile_sem_poison_stack.pop()
            assert popped is self._sem_poison
            nc.clear_and_free_semaphores(
                list(self.sems.allocated().values()) + [semB, semC]
            )

    dt = mybir.dt

    nc = bass.Bass()

    xT = nc.dram_tensor("xT", [DIM, R], dt.bfloat16, kind="ExternalInput")
    yT = nc.dram_tensor("yT", [DIM, R], dt.bfloat16, kind="ExternalInput")
    wxT = nc.dram_tensor("wxT", [DIM, CLASSES], dt.bfloat16, kind="ExternalInput")
    wyT = nc.dram_tensor("wyT", [DIM, CLASSES], dt.bfloat16, kind="ExternalInput")
    bxb = nc.dram_tensor("bxb", [P, CLASSES], dt.bfloat16, kind="ExternalInput")
    byb = nc.dram_tensor("byb", [P, CLASSES], dt.bfloat16, kind="ExternalInput")

    aa_d = nc.dram_tensor("alpha_a", [R, CLASSES], dt.float32, kind="ExternalOutput")
    ax_d = nc.dram_tensor("alpha_x", [R, CLASSES], dt.float32, kind="ExternalOutput")
    ay_d = nc.dram_tensor("alpha_y", [R, CLASSES], dt.float32, kind="ExternalOutput")

    xT3 = xT.rearrange("(ko p) r -> p ko r", p=P)
    yT3 = yT.rearrange("(ko p) r -> p ko r", p=P)
    wxT3 = wxT.rearrange("(ko p) c -> p ko c", p=P)
    wyT3 = wyT.rearrange("(ko p) c -> p ko c", p=P)
    aa3 = aa_d.rearrange("(t p) c -> t p c", p=P)
    ax3 = ax_d.rearrange("(t p) c -> t p c", p=P)
    ay3 = ay_d.rearrange("(t p) c -> t p c", p=P)

    # softplus(x) = ln(exp(x) + 1); the installed ACT tables have no direct
    # softplus, but exp and ln share one table set. Pre-activation values are
    # within +-4 so exp cannot overflow.
    EXP = mybir.ActivationFunctionType.Exp
    LN = mybir.ActivationFunctionType.Ln
    ADD = mybir.AluOpType.add
    MULT = mybir.AluOpType.mult

    HS = [slice(0, NH), slice(NH, CLASSES)]

    with LeanTailTileContext(nc) as tc:
        with (
            tc.tile_pool(name="wpool", bufs=1) as wpool,
            tc.tile_pool(name="xpool", bufs=1) as xpool,
            tc.tile_pool(name="spool", bufs=1) as spool,
            tc.tile_pool(name="opool", bufs=1) as opool,
            tc.tile_pool(name="psum", bufs=1, space="PSUM") as ppool,
        ):
            # -- input DMAs ------------------------------------------------
            # One kk-chunk (2 K slices) per DMA so the PE's wait granularity
            # matches arrival. SP programs the phase-0 ramp in consumption
            # order; Pool programs everything that fires from phase 1 on.
            wx_sb = [[None] * (KCH // 2) for _ in range(2)]   # [h][kk]
            wy_sb = [[None] * (KCH // 2) for _ in range(2)]
            wx_dma = [[None] * (KCH // 2) for _ in range(2)]
            wy_dma = [[None] * (KCH // 2) for _ in range(2)]
            x0_sb, x1_sb, y0_sb, y1_sb = [], [], [], []
            x0_dma, x1_dma, y0_dma, y1_dma = [], [], [], []

            # The first 4 K slices ride single-k DMAs so the very first
            # matmul's data is ~128KB away, not ~512KB; wx ramp on SP and
            # x ramp on Pool program in parallel.
            wx0s_sb, wx0s_dma, x0s_sb, x0s_dma = [], [], [], []
            for k in range(4):
                t_ = wpool.tile([P, 1, NH], dt.bfloat16, tag=f"wx0s_{k}")
                wx0s_dma.append(nc.sync.dma_start(t_[:], wxT3[:, k:k + 1, HS[0]]))
                wx0s_sb.append(t_)
            for kk in range(2, KCH // 2):
                t_ = wpool.tile([P, 2, NH], dt.bfloat16, tag=f"wx0_{kk}")
                wx_dma[0][kk] = nc.sync.dma_start(
                    t_[:], wxT3[:, 2 * kk:2 * kk + 2, HS[0]]
                )
                wx_sb[0][kk] = t_
            for kk in range(KCH // 2):
                t_ = wpool.tile([P, 2, NH], dt.bfloat16, tag=f"wx1_{kk}")
                wx_dma[1][kk] = nc.sync.dma_start(
                    t_[:], wxT3[:, 2 * kk:2 * kk + 2, HS[1]]
                )
                wx_sb[1][kk] = t_
            bx_sb = wpool.tile([P, CLASSES], dt.bfloat16, tag="bx")
            bx_dma = nc.sync.dma_start(bx_sb[:], bxb[:])

            # Pool, in firing order: x0 ramp (phase 0), wy-h0 + y0
            # (phase 1), wy-h1 + x1 (phase 2), y1 (phase 3).
            for k in range(4):
                t_ = xpool.tile([P, 1, 4 * P], dt.bfloat16, tag=f"x0s_{k}")
                x0s_dma.append(nc.gpsimd.dma_start(t_[:], xT3[:, k:k + 1, 0:4 * P]))
                x0s_sb.append(t_)
            for kk in range(2, KCH // 2):
                t_ = xpool.tile([P, 2, 4 * P], dt.bfloat16, tag=f"x0_{kk}")
                x0_dma.append(nc.gpsimd.dma_start(t_[:], xT3[:, 2 * kk:2 * kk + 2, 0:4 * P]))
                x0_sb.append(t_)
            for kk in range(KCH // 2):
                t_ = wpool.tile([P, 2, NH], dt.bfloat16, tag=f"wy0_{kk}")
                wy_dma[0][kk] = nc.gpsimd.dma_start(
                    t_[:], wyT3[:, 2 * kk:2 * kk + 2, HS[0]]
                )
                wy_sb[0][kk] = t_
                t_ = xpool.tile([P, 2, 4 * P], dt.bfloat16, tag=f"y0_{kk}")
                y0_dma.append(nc.gpsimd.dma_start(t_[:], yT3[:, 2 * kk:2 * kk + 2, 0:4 * P]))
                y0_sb.append(t_)
            by_sb = wpool.tile([P, CLASSES], dt.bfloat16, tag="by")
            by_dma = nc.gpsimd.dma_start(by_sb[:], byb[:])
            for kk in range(KCH // 2):
                t_ = wpool.tile([P, 2, NH], dt.bfloat16, tag=f"wy1_{kk}")
                wy_dma[1][kk] = nc.gpsimd.dma_start(
                    t_[:], wyT3[:, 2 * kk:2 * kk + 2, HS[1]]
                )
                wy_sb[1][kk] = t_
                t_ = xpool.tile([P, 2, 4 * P], dt.bfloat16, tag=f"x1_{kk}")
                x1_dma.append(nc.gpsimd.dma_start(t_[:], xT3[:, 2 * kk:2 * kk + 2, 4 * P:R]))
                x1_sb.append(t_)
            for kk in range(KCH // 2):
                t_ = xpool.tile([P, 2, 4 * P], dt.bfloat16, tag=f"y1_{kk}")
                y1_dma.append(nc.gpsimd.dma_start(t_[:], yT3[:, 2 * kk:2 * kk + 2, 4 * P:R]))
                y1_sb.append(t_)

            def x_slice(k, r):  # lhsT for global row tile r, K-chunk k
                if r < 4:
                    if k < 4:
                        return x0s_sb[k][:, 0, r * P:(r + 1) * P]
                    return x0_sb[k // 2 - 2][:, k % 2, r * P:(r + 1) * P]
                return x1_sb[k // 2][:, k % 2, (r - 4) * P:(r - 3) * P]

            def y_slice(k, r):
                if r < 4:
                    return y0_sb[k // 2][:, k % 2, r * P:(r + 1) * P]
                return y1_sb[k // 2][:, k % 2, (r - 4) * P:(r - 3) * P]

            def wx_slice(k, h, cs=slice(0, NH)):
                if h == 0 and k < 4:
                    return wx0s_sb[k][:, 0, cs]
                return wx_sb[h][k // 2][:, k % 2, cs]

            def wy_slice(k, h, cs=slice(0, NH)):
                return wy_sb[h][k // 2][:, k % 2, cs]

            # PE warm-up: the PE idles at a low p-state and ramps with
            # activity; dummy matmuls during the DMA ramp bring it to full
            # clock before the first real matmul.
            wl = spool.tile([P, P], dt.bfloat16, tag="warm_l")
            nc.vector.memset(wl[:], 0)
            wr = spool.tile([P, NH], dt.bfloat16, tag="warm_r")
            nc.gpsimd.memset(wr[:], 0)
            wp = ppool.tile([P, NH], dt.float32, tag="bk7", name="warmp")
            for _ in range(4):
                nc.tensor.matmul(wp[:], wl[:], wr[:], start=True, stop=True)

            # -- compute ---------------------------------------------------
            # Units of [4,2,1,1] row tiles; per unit the phases are
            # X-h0, X-h1, Y-h0, Y-h1, each k-outer over the unit's tiles.
            # Each phase's epilogue overlaps the next phase's matmuls.
            from concourse.tile_rust import add_dep_helper

            UNITS = [(0, 4), (4, 2), (6, 1), (7, 1)]
            mm_anchor = {}  # (phase_idx, k) -> last MM instruction
            bank_ptr = 0    # rotating PSUM bank pointer
            phase_idx = 0

            # t1 holds sp1 = softplus(out_x) per row tile from X epilogue to
            # Y epilogue. Tags: per-jj for U0 (reused by U1), own for U2/U3.
            t1_t = [None] * NT

            def t1_tag(u, r):
                return f"t1_{r % 4}" if u < 2 else f"t1_u{u}"

            for u, (r0, nrt) in enumerate(UNITS):
                last_unit = u == len(UNITS) - 1

                # ---- X phases (h = 0, 1) ----
                psx = [[None] * nrt for _ in range(2)]
                for h in range(2):
                    for jj in range(nrt):
                        b = (bank_ptr + jj) % 8
                        psx[h][jj] = ppool.tile(
                            [P, NH], dt.float32, tag=f"bk{b}", name=f"xu{u}h{h}j{jj}"
                        )
                    bank_ptr += nrt
                    for k in range(KCH):
                        st, sp = k == 0, k == KCH - 1
                        for jj in range(nrt):
                            mm = nc.tensor.matmul(
                                psx[h][jj][:], x_slice(k, r0 + jj), wx_slice(k, h),
                                start=st, stop=sp,
                            )
                        mm_anchor[(phase_idx, k)] = mm.ins
                    phase_idx += 1

                    # h epilogue (overlaps the next phase's matmuls):
                    # sp1 = softplus(psum + bx) -> t1; ax = sp1+1 -> DMA.
                    for jj in range(nrt):
                        r = r0 + jj
                        if h == 0:
                            t1_t[r] = spool.tile(
                                [P, CLASSES], dt.float32, tag=t1_tag(u, r),
                                name=f"t1_{r}",
                            )
                        hs = HS[h]
                        sp1 = t1_t[r][:, hs]
                        nc.vector.tensor_tensor(sp1, psx[h][jj][:], bx_sb[:, hs], ADD)
                        nc.scalar.activation(sp1, sp1, EXP)
                        nc.scalar.activation(sp1, sp1, LN, bias=1.0)
                        axt = opool.tile([P, CLASSES], dt.float32, tag=f"ax_{jj % 2}")
                        nc.vector.tensor_scalar_add(axt[:, hs], sp1, 1.0)
                        nc.scalar.dma_start(ax3[r][:, hs], axt[:, hs])

                # ---- Y phases ----
                def y_epilogue(r, psum, cs, t2, ay, aa, wslice_unused=None):
                    # sp2 = softplus(psum + by); ay = sp2+1 (ACT) in
                    # parallel with aa = sp1*(sp2/C + 1) + ay (DVE).
                    sp2 = t2[:, cs]
                    nc.vector.tensor_tensor(sp2, psum, by_sb[:, cs], ADD)
                    nc.scalar.activation(sp2, sp2, EXP)
                    nc.scalar.activation(sp2, sp2, LN, bias=1.0)
                    nc.scalar.add(ay[:, cs], sp2, 1.0)
                    nc.scalar.dma_start(ay3[r][:, cs], ay[:, cs])
                    w2 = aa[:, cs]
                    nc.vector.tensor_scalar(w2, sp2, 1.0 / CLASSES, 1.0, MULT, ADD)
                    nc.vector.tensor_tensor(w2, w2, t1_t[r][:, cs], MULT)
                    nc.vector.tensor_tensor(w2, w2, ay[:, cs], ADD)
                    nc.sync.dma_start(aa3[r][:, cs], aa[:, cs])

                if last_unit:
                    # Quarter-split with per-quarter epilogue so only a
                    # 250-wide chain is exposed after the last matmul.
                    r = r0
                    t2 = opool.tile([P, CLASSES], dt.float32, tag="t2_0")
                    ay = opool.tile([P, CLASSES], dt.float32, tag="ay_0")
                    aa = opool.tile([P, CLASSES], dt.float32, tag="aa_0")
                    SEGS = [(0, 250), (250, 250), (500, 250), (750, 125), (875, 125)]
                    for q, (s0, w) in enumerate(SEGS):
                        qs = slice(s0, s0 + w)
                        h, off = s0 // NH, s0 % NH
                        b = bank_ptr % 8
                        bank_ptr += 1
                        psq = ppool.tile([P, w], dt.float32, tag=f"bk{b}", name=f"yq{q}")
                        wqs = slice(off, off + w)
                        for k in range(KCH):
                            st, sp = k == 0, k == KCH - 1
                            mm = nc.tensor.matmul(
                                psq[:], y_slice(k, r), wy_slice(k, h, wqs),
                                start=st, stop=sp,
                            )
                        mm_anchor[(phase_idx, k)] = mm.ins
                        phase_idx += 1
                        y_epilogue(r, psq[:], qs, t2, ay, aa)
                    continue

                psy = [[None] * nrt for _ in range(2)]
                for h in range(2):
                    for jj in range(nrt):
                        b = (bank_ptr + jj) % 8
                        psy[h][jj] = ppool.tile(
                            [P, NH], dt.float32, tag=f"bk{b}", name=f"yu{u}h{h}j{jj}"
                        )
                    bank_ptr += nrt
                    for k in range(KCH):
                        st, sp = k == 0, k == KCH - 1
                        for jj in range(nrt):
                            mm = nc.tensor.matmul(
                                psy[h][jj][:], y_slice(k, r0 + jj), wy_slice(k, h),
                                start=st, stop=sp,
                            )
                        mm_anchor[(phase_idx, k)] = mm.ins
                    phase_idx += 1

                    for jj in range(nrt):
                        r = r0 + jj
                        t2 = opool.tile([P, CLASSES], dt.float32, tag=f"t2_{jj % 2}")
                        ay = opool.tile([P, CLASSES], dt.float32, tag=f"ay_{jj % 2}")
                        aa = opool.tile([P, CLASSES], dt.float32, tag=f"aa_{jj % 2}")
                        y_epilogue(r, psy[h][jj][:], HS[h], t2, ay, aa)

            # -- DMA backpressure: gate transfers on PE progress -----------
            # Phase ids: U0: 0=X-h0 1=X-h1 2=Y-h0 3=Y-h1; U1: 4..7;
            # U2: 8..11; U3: 12,13 then quarters 14..17.
            def _gate(dma, phase, k, why):
                add_dep_helper(
                    dma.ins, mm_anchor[(phase, min(max(k, 0), KCH - 1))], reason=why
                )

            for kk in range(KCH // 2):
                _gate(wx_dma[1][kk], 0, 2 * kk, "wx h1 stage")
                _gate(wy_dma[0][kk], 1, 2 * kk, "wy h0 stage")
                _gate(y0_dma[kk], 1, 2 * kk, "y u0 stage")
                _gate(wy_dma[1][kk], 2, 2 * kk, "wy h1 stage")
                _gate(x1_dma[kk], 2, 2 * kk, "x rest stage")
                _gate(y1_dma[kk], 3, 2 * kk, "y rest stage")
            _gate(bx_dma, 0, 10, "bias x stage")
            _gate(by_dma, 1, 10, "bias y stage")

    _split_waits(nc)
    return nc


def kernel(x, y, Wx, bx, Wy, by):
    global LAST_RESULTS
    from concourse.bass_utils import run_bass_kernel_spmd

    if "nc" not in _CACHE:
        _CACHE["nc"] = _build_nc()
    nc = _CACHE["nc"]

    bf16 = ml_dtypes.bfloat16
    x = np.asarray(x, dtype=np.float32)
    y = np.asarray(y, dtype=np.float32)
    xb = x.astype(bf16)                       # [BATCH, DIM]
    yb = y.astype(bf16)
    wxT = np.ascontiguousarray(np.asarray(Wx, dtype=np.float32).astype(bf16).T)  # [DIM, CLASSES]
    wyT = np.ascontiguousarray(np.asarray(Wy, dtype=np.float32).astype(bf16).T)
    bxb = np.ascontiguousarray(
        np.broadcast_to(np.asarray(bx, dtype=np.float32).astype(bf16), (P, CLASSES))
    )
    byb = np.ascontiguousarray(
        np.broadcast_to(np.asarray(by, dtype=np.float32).astype(bf16), (P, CLASSES))
    )

    xTb = np.ascontiguousarray(xb.T)          # [DIM, BATCH]
    yTb = np.ascontiguousarray(yb.T)

    in_maps = []
    for c in range(NCORES):
        rs = slice(c * R, (c + 1) * R)
        in_maps.append(
            {
                "xT": np.ascontiguousarray(xTb[:, rs]),
                "yT": np.ascontiguousarray(yTb[:, rs]),
                "wxT": wxT,
                "wyT": wyT,
                "bxb": bxb,
                "byb": byb,
            }
        )

    res = run_bass_kernel_spmd(nc, in_maps, core_ids=list(range(NCORES)))
    LAST_RESULTS = res

    aa = np.concatenate([res.results[c]["alpha_a"] for c in range(NCORES)], axis=0)
    ax = np.concatenate([res.results[c]["alpha_x"] for c in range(NCORES)], axis=0)
    ay = np.concatenate([res.results[c]["alpha_y"] for c in range(NCORES)], axis=0)
    return (aa, ax, ay)
